# revision 18
# baseline (speedup 1.0000x reference)
import sys
import numpy as np

for _p in ("/opt/trn_rl_repo",):
    if _p not in sys.path:
        sys.path.insert(0, _p)

# ---- hardcoded problem shape (nn_A_MPNCOV): x [1024, 128, 14, 14] fp32 ----
B_TOT = 1024
D = 128
M = 196          # h*w
MP = 256         # M padded to 2 partition tiles
N_CORES = 8
NI = B_TOT // N_CORES   # 128 items per core
NB = 16                 # items per block
NBLK = NI // NB         # 8 blocks

_NC_CACHE = {}


def _build_nc():
    if "nc" in _NC_CACHE:
        return _NC_CACHE["nc"]
    import concourse.bass as bass
    import concourse.bass_isa as bass_isa
    import concourse.mybir as mybir
    from concourse.tile import TileContext

    f32 = mybir.dt.float32
    bf16 = mybir.dt.bfloat16
    AF = mybir.ActivationFunctionType

    nc = bass.Bass()
    xt = nc.dram_tensor("xt", [NI, MP, D], bf16, kind="ExternalInput")
    i15 = nc.dram_tensor("i15", [D, D], bf16, kind="ExternalInput")
    i3n = nc.dram_tensor("i3n", [D, D], bf16, kind="ExternalInput")
    ib = nc.dram_tensor("ib", [D, D], bf16, kind="ExternalInput")
    yout = nc.dram_tensor("yout", [NI, D, D], f32, kind="ExternalOutput")
    cscr = nc.dram_tensor("cscr", [NBLK, 32], f32, kind="Internal")

    with TileContext(nc) as tc:
        with (
            tc.tile_pool(name="consts", bufs=1) as consts,
            tc.tile_pool(name="xin", bufs=2) as xin,
            tc.tile_pool(name="blk", bufs=2) as blk,
            tc.tile_pool(name="mats", bufs=2) as mats,
            tc.tile_pool(name="outp", bufs=2) as outp,
            tc.tile_pool(name="psS", bufs=1, space="PSUM") as psSp,
            tc.tile_pool(name="psns", bufs=2, space="PSUM") as psnsp,
            tc.tile_pool(name="psyz", bufs=2, space="PSUM") as psyzp,
        ):
            sb_i15 = consts.tile([D, D], bf16)
            nc.sync.dma_start(out=sb_i15, in_=i15[:, :])
            sb_i3n = consts.tile([D, D], bf16)
            nc.sync.dma_start(out=sb_i3n, in_=i3n[:, :])
            sb_ib = consts.tile([D, D], bf16)
            nc.sync.dma_start(out=sb_ib, in_=ib[:, :])

            for blki in range(NBLK):
                b0 = blki * NB
                XA = xin.tile([D, NB * D], bf16, tag="XA")
                XB = xin.tile([D, NB * D], bf16, tag="XB")
                nc.gpsimd.dma_start(
                    out=XA.rearrange("p (n e) -> p n e", n=NB),
                    in_=xt[b0 : b0 + NB, 0:D, :].rearrange("n m e -> m n e"),
                )
                nc.gpsimd.dma_start(
                    out=XB.rearrange("p (n e) -> p n e", n=NB),
                    in_=xt[b0 : b0 + NB, D : 2 * D, :].rearrange("n m e -> m n e"),
                )

                XA2 = xin.tile([D, NB * D], bf16, tag="XA2")
                XB2 = xin.tile([D, NB * D], bf16, tag="XB2")
                nc.gpsimd.tensor_copy(XA2, XA)
                nc.gpsimd.tensor_copy(XB2, XB)

                # per-block tiles
                RSA = blk.tile([D, NB], f32, tag="RSA")
                RSB = blk.tile([D, NB], f32, tag="RSB")
                trrow = blk.tile([1, NB], f32, tag="trrow")
                crow = blk.tile([1, 32], f32, tag="crow")
                bc = blk.tile([D, 32], f32, tag="bc")

                # 4 bank-tiles, each holds 4 items' S
                psS_banks = [
                    psSp.tile([D, 512], f32, tag=f"psSb{j}", name=f"psSb{j}")
                    for j in range(4)
                ]

                # ---- PASS A: gram + rowsq matmuls ----
                for i in range(NB):
                    sl = slice(i * D, (i + 1) * D)
                    XAi = XA2[:, sl]
                    XBi = XB2[:, sl]
                    psS = psS_banks[i // 4][:, (i % 4) * D : (i % 4 + 1) * D]
                    dmp = mats.tile([D, D], bf16, tag="dump")
                    nc.scalar.activation(
                        dmp, XAi, AF.Square, accum_out=RSA[:, i : i + 1]
                    )
                    dmp2 = mats.tile([D, D], bf16, tag="dump")
                    nc.scalar.activation(
                        dmp2, XBi, AF.Square, accum_out=RSB[:, i : i + 1]
                    )
                    nc.tensor.matmul(psS, XAi, XAi, start=True, stop=False)
                    nc.tensor.matmul(psS, XBi, XBi, start=False, stop=True)

                # ---- block scalar phase ----
                # tr(S)_i = sum rowsq ; w = tr(S); c1 = 1/w ; c2 = sqrt(w/196)
                nc.vector.tensor_add(RSA, RSA, RSB)
                nc.gpsimd.tensor_reduce(
                    out=trrow, in_=RSA, axis=mybir.AxisListType.C,
                    op=mybir.AluOpType.add,
                )
                nc.vector.reciprocal(crow[:, 0:NB], trrow)
                nc.scalar.activation(
                    crow[:, 16 : 16 + NB], trrow, AF.Sqrt, scale=1.0 / 196.0
                )
                nc.gpsimd.dma_start(out=cscr[blki : blki + 1, :], in_=crow)
                _crow_d = cscr[blki : blki + 1, :]
                nc.gpsimd.dma_start(
                    out=bc,
                    in_=bass.AP(tensor=_crow_d.tensor, offset=_crow_d.offset,
                                ap=[[0, D], [1, 32]]),
                )

                OUT = outp.tile([D, NB * D], f32, tag="OUT")

                # ---- PASS B: Newton-Schulz per item ----
                for i in range(NB):
                    sl = slice(i * D, (i + 1) * D)
                    psS = psS_banks[i // 4][:, (i % 4) * D : (i % 4 + 1) * D]
                    A = mats.tile([D, D], bf16, tag="A")
                    nc.scalar.mul(A, psS, bc[:, i : i + 1])
                    Ah = mats.tile([D, D], bf16, tag="Ah")
                    nc.vector.tensor_scalar_mul(Ah, A, -0.5)
                    YZ1 = mats.tile([D, 2 * D], bf16, tag="YZ1")
                    nc.vector.tensor_add(YZ1[:, D : 2 * D], sb_i15, Ah)  # Z1
                    psP = psnsp.tile([D, D], f32, tag="nsps")
                    nc.tensor.matmul(psP, A, A, start=True, stop=False)
                    nc.tensor.matmul(psP, sb_i3n, A, start=False, stop=True)  # A^2-3A
                    nc.scalar.mul(YZ1[:, 0:D], psP, -0.5)  # Y1
                    psT2 = psnsp.tile([D, D], f32, tag="nsps")
                    nc.tensor.matmul(psT2, YZ1[:, D : 2 * D], YZ1[:, 0:D], start=True, stop=False)
                    nc.tensor.matmul(psT2, sb_i3n, sb_ib, start=False, stop=True)  # Z1Y1-3I
                    ZY2 = mats.tile([D, D], bf16, tag="ZY2")
                    nc.vector.tensor_scalar_mul(ZY2, psT2, -0.5)
                    psYZ = psyzp.tile([D, 2 * D], f32, tag="psYZ")
                    nc.tensor.matmul(psYZ, ZY2, YZ1, start=True, stop=True)  # [Y2|Z2]
                    YZ2 = mats.tile([D, 2 * D], bf16, tag="YZ2")
                    nc.vector.tensor_copy(YZ2, psYZ)
                    psT3 = psnsp.tile([D, D], f32, tag="nsps")
                    nc.tensor.matmul(psT3, YZ2[:, D : 2 * D], YZ2[:, 0:D], start=True, stop=False)
                    nc.tensor.matmul(psT3, sb_i3n, sb_ib, start=False, stop=True)  # Z2Y2-3I
                    ZY3 = mats.tile([D, D], bf16, tag="ZY3")
                    nc.scalar.mul(ZY3, psT3, -0.5)
                    psY3 = psnsp.tile([D, D], f32, tag="nsps")
                    nc.tensor.matmul(psY3, ZY3, YZ2[:, 0:D], start=True, stop=True)
                    nc.scalar.mul(OUT[:, sl], psY3, bc[:, 16 + i : 17 + i])

                nc.sync.dma_start(
                    out=yout[b0 : b0 + NB, :, :].rearrange("n d e -> d n e"),
                    in_=OUT.rearrange("p (n e) -> p n e", n=NB),
                )

    # this walrus build accepts at most ONE sync-wait per instruction; hoist
    # extra waits onto standalone same-engine EventSemaphore carriers.
    nsplit = 0
    for b in nc.m.functions[0].blocks:
        out = []
        for inst in b.instructions:
            si = inst.sync_info
            tname = type(inst).__name__
            keep = 0 if ("ISA" in tname or "PartitionAllReduce" in tname) else 1
            if si is not None and si.on_wait and len(si.on_wait) > keep:
                waits = list(si.on_wait)
                split, kept = (waits, []) if keep == 0 else (waits[:-1], [waits[-1]])
                for w in split:
                    nsplit += 1
                    car = mybir.InstEventSemaphore(
                        name=f"WSPLIT-{nsplit}", ins=[], outs=[]
                    )
                    car.engine = inst.engine
                    car.sync_info = mybir.SyncInfo(on_wait=[w], on_update=[])
                    out.append(car)
                inst.sync_info = mybir.SyncInfo(
                    on_wait=kept, on_update=list(si.on_update or [])
                )
            out.append(inst)
        b.instructions = out

    _NC_CACHE["nc"] = nc
    return nc


def kernel(x):
    import ml_dtypes
    from concourse.bass_utils import run_bass_kernel_spmd

    x = np.asarray(x)
    assert x.shape == (B_TOT, D, 14, 14)
    xr = x.reshape(B_TOT, D, M).transpose(0, 2, 1)  # [B, 196, 128]
    xr = xr - xr.mean(axis=1, keepdims=True)
    xp = np.zeros((B_TOT, MP, D), np.float32)
    xp[:, :M, :] = xr
    xb = xp.astype(ml_dtypes.bfloat16)

    eye = np.eye(D, dtype=np.float32)
    i15 = (1.5 * eye).astype(ml_dtypes.bfloat16)
    i3n = (-3.0 * eye).astype(ml_dtypes.bfloat16)
    ib = eye.astype(ml_dtypes.bfloat16)

    nc = _build_nc()
    in_maps = [
        {
            "xt": np.ascontiguousarray(xb[c * NI : (c + 1) * NI]),
            "i15": i15,
            "i3n": i3n,
            "ib": ib,
        }
        for c in range(N_CORES)
    ]
    res = run_bass_kernel_spmd(nc, in_maps, core_ids=list(range(N_CORES)))
    y = np.concatenate([r["yout"] for r in res.results], axis=0)  # [B, 128, 128]

    r_, c_ = np.triu_indices(D)
    flat = r_ * D + c_
    return np.ascontiguousarray(y.reshape(B_TOT, D * D)[:, flat][..., None]).astype(
        np.float32
    )
